# revision 1
# baseline (speedup 1.0000x reference)
"""v7: row-flat layout, all-static DMAs, 2-dim DRAM access patterns.

Problem: x [64, 3, 512, 512] f32, shifts [64, 2] int32 in [-16, 16].
out[b, c, h, w] = x[b, c, (h - shifts[b,0]) % 512, (w - shifts[b,1]) % 512]

Pure data parallel over batch (8 per core). Per batch:
- H-roll applied on the DRAM->SBUF load through a 33-case If chain (sync/SP
  engine). Every load box has a fully CONTIGUOUS DRAM side (consecutive rows
  of one channel), so nothing gets compiler-unrolled.
- W-roll applied on the SBUF->DRAM store through a 33-case If chain (scalar/
  ACT engine, its own HWDGE ring). The SBUF row-flat layout makes the DRAM
  side a uniform [1536 rows x w-slice] 2-dim pattern - native PDMA2D shape.

SBUF layout per slot: [128 partitions, 12, 512]: global row r = c*512 + h
lives at partition r // 12, free slot r % 12. (Any DMA whose DRAM-side AP
needs >2 dims is statically unrolled ~128x by the compiler - that's what
makes naive layouts take an hour to compile.)

Cross-engine pipelining uses per-slot semaphores (a single counting
semaphore would be ambiguous: batch b+1 completions could satisfy batch b's
wait because DMA completions are unordered).
"""

from contextlib import ExitStack

import numpy as np

import concourse.bass as bass
import concourse.mybir as mybir
from concourse.bass_utils import run_bass_kernel_spmd

B_TOTAL, C, H, W = 64, 3, 512, 512
N_CORES = 8
B = B_TOTAL // N_CORES
MAX_SHIFT = 16
P = 128
J = (C * H) // P  # 12 rows per partition; r = c*512 + h = p*J + j
NBUF = 4

LOADS_PER_BATCH = 18  # semaphore-equalized
STORES_PER_BATCH = 2


def _copy_rows(sync, tile_s, r0, x_src, load_sem):
    """Copy len(src rows) contiguous DRAM rows into tile rows [r0, r0+n).

    x_src: DRAM AP [n, 512] (contiguous rows of one channel).
    Emits 1-3 DMAs (partial head partition / full body / partial tail).
    Returns DMA count.
    """
    n = x_src.shape[0]
    cnt = 0
    lo = r0
    hi = r0 + n
    src = 0
    if lo % J != 0 and lo < hi:
        m = min(hi - lo, J - lo % J)
        p = lo // J
        sync.dma_start(
            tile_s[p : p + 1, (lo % J) * W : (lo % J + m) * W],
            x_src[src : src + m].rearrange("r w -> (r w)")[None, :],
        ).then_inc(load_sem, 16)
        cnt += 1
        lo += m
        src += m
    nfull = (hi - lo) // J
    if nfull > 0:
        sync.dma_start(
            tile_s[lo // J : lo // J + nfull, :],
            x_src[src : src + nfull * J].rearrange("(p q) w -> p (q w)", q=J),
        ).then_inc(load_sem, 16)
        cnt += 1
        lo += nfull * J
        src += nfull * J
    if lo < hi:
        m = hi - lo
        p = lo // J
        sync.dma_start(
            tile_s[p : p + 1, 0 : m * W],
            x_src[src : src + m].rearrange("r w -> (r w)")[None, :],
        ).then_inc(load_sem, 16)
        cnt += 1
    return cnt


def _emit_loads(sync, x, tile_s, b, hoff, load_sem):
    """tile row (c*512 + h) = x[b, c, (h + hoff) % 512, :]."""
    n = 0
    if hoff == 0:
        for c in range(C):
            n += _copy_rows(sync, tile_s, c * H, x[b, c, :, :], load_sem)
    else:
        R = H - hoff
        for c in range(C):
            # piece 1: dst rows [c*512, c*512+R) <- src h [hoff, 512)
            n += _copy_rows(sync, tile_s, c * H, x[b, c, hoff:H, :], load_sem)
            # piece 2: dst rows [c*512+R, c*512+512) <- src h [0, hoff)
            n += _copy_rows(sync, tile_s, c * H + R, x[b, c, 0:hoff, :], load_sem)
    assert n <= LOADS_PER_BATCH, (hoff, n)
    if n < LOADS_PER_BATCH:
        sync.sem_inc(load_sem, 16 * (LOADS_PER_BATCH - n))
    return n


def _emit_stores(nc, scalar, out, tile_s, b, woff, store_sem):
    """out[b, c, h, w] = tile[c*512+h, (w + woff) % 512]."""
    out_rw = out[b].rearrange("c h w -> (c h) w")  # [1536, 512] uniform stride
    tile_j = tile_s.rearrange("p (j w) -> p j w", w=W)
    if woff == 0:
        scalar.dma_start(out_rw, tile_s[:, :]).then_inc(store_sem, 16)
        n = 1
    else:
        with nc.allow_non_contiguous_dma(
            reason="W-roll wrap strip can be a single column"
        ):
            # box D: out[.., 0:W-woff] = tile[.., woff:W]
            scalar.dma_start(
                out_rw[:, 0 : W - woff], tile_j[:, :, woff:W]
            ).then_inc(store_sem, 16)
            # box E: out[.., W-woff:W] = tile[.., 0:woff]
            scalar.dma_start(
                out_rw[:, W - woff : W], tile_j[:, :, 0:woff]
            ).then_inc(store_sem, 16)
        n = 2
    if n < STORES_PER_BATCH:
        scalar.sem_inc(store_sem, 16 * (STORES_PER_BATCH - n))
    return n


def build_kernel():
    nc = bass.Bass()
    x = nc.dram_tensor("x", [B, C, H, W], mybir.dt.float32, kind="ExternalInput")
    shifts = nc.dram_tensor("shifts", [B, 2], mybir.dt.int32, kind="ExternalInput")
    out = nc.dram_tensor("out", [B, C, H, W], mybir.dt.float32, kind="ExternalOutput")

    with (
        nc.sbuf_tensor([P, NBUF, J * W], mybir.dt.float32) as tiles,
        nc.sbuf_tensor([1, B * 2], mybir.dt.int32) as sb_shifts,
        nc.semaphore("pre_sem") as pre_sem,
        ExitStack() as stack,
    ):
        load_sems = [
            stack.enter_context(nc.semaphore(f"load_sem{s}")) for s in range(NBUF)
        ]
        store_sems = [
            stack.enter_context(nc.semaphore(f"store_sem{s}")) for s in range(NBUF)
        ]
        block = stack.enter_context(nc.Block())

        @block.sync
        def _(sync):
            sync.dma_start(
                sb_shifts[0:1, :], shifts.rearrange("b s -> (b s)")[None, :]
            ).then_inc(pre_sem, 16)
            sync.wait_ge(pre_sem, 16)
            with sync.register("r_sh") as r_sh:
                for b in range(B):
                    s = b % NBUF
                    if b >= NBUF:
                        sync.wait_ge(
                            store_sems[s], 16 * STORES_PER_BATCH * (b // NBUF)
                        )
                    sync.reg_load(r_sh, sb_shifts[0:1, 2 * b : 2 * b + 1])
                    sh = sync.snap(r_sh)
                    for v in range(-MAX_SHIFT, MAX_SHIFT + 1):
                        with sync.If(sh == v):
                            _emit_loads(
                                sync, x, tiles[:, s], b, (-v) % H, load_sems[s]
                            )

        @block.scalar
        def _(scalar):
            scalar.wait_ge(pre_sem, 16)
            with scalar.register("r_sw") as r_sw:
                for b in range(B):
                    s = b % NBUF
                    scalar.wait_ge(
                        load_sems[s], 16 * LOADS_PER_BATCH * (b // NBUF + 1)
                    )
                    scalar.reg_load(r_sw, sb_shifts[0:1, 2 * b + 1 : 2 * b + 2])
                    sw = scalar.snap(r_sw)
                    for v in range(-MAX_SHIFT, MAX_SHIFT + 1):
                        with scalar.If(sw == v):
                            _emit_stores(
                                nc,
                                scalar,
                                out,
                                tiles[:, s],
                                b,
                                (-v) % W,
                                store_sems[s],
                            )
            for s in range(NBUF):
                uses = (B - s + NBUF - 1) // NBUF
                scalar.wait_ge(store_sems[s], 16 * STORES_PER_BATCH * uses)

    return nc


_NC_CACHE = None


def _get_nc():
    global _NC_CACHE
    if _NC_CACHE is None:
        _NC_CACHE = build_kernel()
    return _NC_CACHE


def kernel(x: np.ndarray, shifts: np.ndarray) -> np.ndarray:
    assert x.shape == (B_TOTAL, C, H, W), x.shape
    assert shifts.shape == (B_TOTAL, 2), shifts.shape
    x = np.ascontiguousarray(x, dtype=np.float32)
    shifts = np.ascontiguousarray(shifts, dtype=np.int32)

    in_maps = [
        {"x": x[i * B : (i + 1) * B], "shifts": shifts[i * B : (i + 1) * B]}
        for i in range(N_CORES)
    ]
    res = run_bass_kernel_spmd(_get_nc(), in_maps, list(range(N_CORES)))
    return np.concatenate(
        [res.results[i]["out"] for i in range(N_CORES)], axis=0
    ).astype(np.float32)

